# revision 3
# baseline (speedup 1.0000x reference)
"""Trainium2 Bass kernel for nn_Capsule_16484084482446.

Reference math collapses: with cw = softmax(rw, axis=1),
  outputs[b,j,d] = sum_i sum_n cw[b,i,n] * u[b,j,n,d]
                 = sum_n u[b,j,n,d]           (since sum_i cw[b,i,n] == 1)
so the routing loop is a no-op and the final result is
  out = (sum_n x[b,n,:]) @ W   reshaped to (B, 10, 16).

Kernel strategy (data-parallel over batch, 4 batches per core x 8 cores):
  per core: x_shard (4, 4096, 128) viewed as 128 partitions x (128 rows x 128 d);
  partition p holds rows [128p, 128p+128), so batch b owns partitions [32b, 32b+32).

v2b (baseline was fp32 single-queue at 39.9us; trace showed DMA engines
saturated 8.5->31us but a ~3us DVE fold backlog + ~4us fp32 finish chain):
  1. x chunks split across BOTH HWDGE queues (even chunks on sync/qSP,
     odd on scalar/qAct) so descriptor generation overlaps transfers and
     the 16 shared DMA engines never wait on a single queue's DGE.
  2. DVE fold level 1 writes fp16 (fp32+fp32->fp16 add); levels 2+ run
     at the DVE's 2-elem/cycle 16-bit rate. Fold work drops ~21us->~12us
     so the fold never backlogs behind the DMA stream.
  3. gpsimd (idle otherwise) folds two middle chunks to keep the DVE
     ahead of the stream; separate ready-semaphores per folder engine.
  4. All matmuls fp16 single-pass (fp32 is dual-pass on the PE: measured
     1.33us vs ~0.65us per mask-matmul). PSUM accumulates fp32.
  5. W loads early on the sync queue and is cast to fp16 off the
     critical path (baseline loaded W last, stealing x-stream bandwidth).
  6. Chunk sizes taper 16->8->4->2 so the final fold is tiny.
  7. No wait on the out-DMA completion semaphore: the engine drain at
     block exit flushes the queue, overlapping the exit barrier with the
     out-DMA flight.
  Accuracy: fp16 folds of ~N(0,1) data with fp32 PSUM accumulation ->
  rel err ~1e-3 (gate 2e-2; inputs are fixed/seeded, so local pass ==
  harness pass).

Raw Bass (no TileContext): Tile's tail drain needs more sync-wait slots than the
TRN2 CTRL encoding allows for this DMA-lane mix, and its end-of-kernel barriers
would dominate a ~37 us kernel. Every semaphore is cleared by its final consumer
right after its last wait, so the NEFF re-executes cleanly (profilers loop it).
"""

from contextlib import ExitStack

import numpy as np

import concourse.bass as bass
from concourse import mybir
from concourse.bass_utils import run_bass_kernel_spmd

N_CORES = 8
B, N, DIN = 32, 4096, 128
BSH = B // N_CORES          # 4 batches per core
DOUT = 160                  # 10 capsules * 16 dims
# rows-per-partition split: 8-row ramp (early DVE start), 16-row middle,
# tapered tail so the last folds are tiny
CHUNKS = [8, 8, 16, 16, 16, 16, 16, 8, 8, 8, 4, 2, 2]
GP_CHUNKS = (5, 6)          # folded by gpsimd; the rest by DVE
assert sum(CHUNKS) == BSH * N // 128
NCHUNK = len(CHUNKS)
DVE_CHUNKS = [c for c in range(NCHUNK) if c not in GP_CHUNKS]

F32 = mybir.dt.float32
F16 = mybir.dt.float16

_cache = {}


def _fold(eng, xc_c, xh_c, rows):
    """Halving fold of xc_c (fp32, rows*DIN) into xh_c[:, :DIN] (fp16).
    Level 1 casts fp32->fp16; later levels run at the 2x 16-bit rate."""
    s = rows // 2
    op = eng.tensor_add(
        xh_c[:, : s * DIN], xc_c[:, : s * DIN], xc_c[:, s * DIN :]
    )
    while s > 1:
        s //= 2
        op = eng.tensor_add(
            xh_c[:, : s * DIN],
            xh_c[:, : s * DIN],
            xh_c[:, s * DIN : 2 * s * DIN],
        )
    return op


def _build_nc(chunks=None, gp_chunks=None, out_wait=False):
    global CHUNKS, NCHUNK, GP_CHUNKS, DVE_CHUNKS
    if chunks is not None:
        CHUNKS = chunks
    if gp_chunks is not None:
        GP_CHUNKS = gp_chunks
    NCHUNK = len(CHUNKS)
    DVE_CHUNKS = [c for c in range(NCHUNK) if c not in GP_CHUNKS]
    assert sum(CHUNKS) == BSH * N // 128
    nc = bass.Bass()
    x = nc.dram_tensor("x", [BSH, N, DIN], F32, kind="ExternalInput")
    w = nc.dram_tensor("W", [DIN, DOUT], F32, kind="ExternalInput")
    out = nc.dram_tensor("out", [BSH, DOUT], F32, kind="ExternalOutput")

    # (128, 128, 128): partition p, row-in-partition n, feature d
    x3 = x[:].flatten_outer_dims().rearrange("(p n) d -> p n d", p=128)
    starts = np.cumsum([0] + CHUNKS).tolist()

    with ExitStack() as ctx:
        ec = ctx.enter_context
        xc = [ec(nc.sbuf_tensor(f"xc{c}", [128, CHUNKS[c] * DIN], F32))
              for c in range(NCHUNK)]
        xh = [ec(nc.sbuf_tensor(f"xh{c}", [128, (CHUNKS[c] // 2) * DIN], F16))
              for c in range(NCHUNK)]
        w_sb = ec(nc.sbuf_tensor("w_sb", [DIN, DOUT], F32))
        w16 = ec(nc.sbuf_tensor("w16", [DIN, DOUT], F16))
        mask_sb = ec(nc.sbuf_tensor("mask_sb", [128, BSH], F16))
        s16 = ec(nc.sbuf_tensor("s16", [DIN, BSH], F16))
        out_sb = ec(nc.sbuf_tensor("out_sb", [BSH, DOUT], F32))
        psum_s = ec(nc.psum_tensor("psum_s", [DIN, BSH], F32))
        psum_o = ec(nc.psum_tensor("psum_o", [BSH, DOUT], F32))

        dma_w = ec(nc.semaphore("dma_w"))
        dma_c = [ec(nc.semaphore(f"dma_c{c}")) for c in range(NCHUNK)]
        v_red = ec(nc.semaphore("v_red"))    # +1 per finished DVE fold
        g_red = ec(nc.semaphore("g_red"))    # +1 per finished gpsimd fold
        v_w16 = ec(nc.semaphore("v_w16"))    # w16 ready
        pe_sem = ec(nc.semaphore("pe_sem"))
        v_sem = ec(nc.semaphore("v_sem"))    # s16 ready
        v_out = ec(nc.semaphore("v_out"))
        dma_out = ec(nc.semaphore("dma_out"))  # never waited (drain flushes)
        # Sem hygiene without an entry barrier: every semaphore is cleared by
        # its final consumer right after the consumer's last wait on it, so
        # every run (the profiler re-executes the NEFF) starts from zeros.
        # dma_out only ever grows; nothing waits on an absolute value.
        block = ec(nc.Block())

        @block.sync
        def _(sync):
            # W first (needed only at the end, but the queue is free and the
            # first x descriptors come from the scalar queue concurrently)
            sync.dma_start(w_sb[:], w[:]).then_inc(dma_w, 16)
            for c in range(0, NCHUNK, 2):
                sync.dma_start(
                    xc[c][:], x3[:, starts[c] : starts[c + 1], :]
                ).then_inc(dma_c[c], 16)
            sync.wait_ge(v_out, 1)
            sync.sem_clear(v_out)
            sync.dma_start(out[:], out_sb[:]).then_inc(dma_out, 16)
            if out_wait:
                sync.wait_ge(dma_out, 16)
                sync.sem_clear(dma_out)

        @block.scalar
        def _(scalar):
            for c in range(1, NCHUNK, 2):
                scalar.dma_start(
                    xc[c][:], x3[:, starts[c] : starts[c + 1], :]
                ).then_inc(dma_c[c], 16)

        @block.gpsimd
        def _(gpsimd):
            for c in GP_CHUNKS:
                gpsimd.wait_ge(dma_c[c], 16)
                gpsimd.sem_clear(dma_c[c])
                _fold(gpsimd, xc[c], xh[c], CHUNKS[c]).then_inc(g_red, 1)

        @block.vector
        def _(vector):
            # 0/1 batch mask, one 32-partition quadrant at a time (nonzero
            # partition bases only allow 32-partition windows)
            for q in range(4):
                for b in range(BSH):
                    vector.memset(
                        mask_sb[32 * q : 32 * (q + 1), b : b + 1],
                        1.0 if q == b else 0.0,
                    )
            vector.wait_ge(dma_w, 16)
            vector.sem_clear(dma_w)
            vector.tensor_copy(w16[:], w_sb[:]).then_inc(v_w16, 1)
            for c in DVE_CHUNKS:
                vector.wait_ge(dma_c[c], 16)
                vector.sem_clear(dma_c[c])
                _fold(vector, xc[c], xh[c], CHUNKS[c]).then_inc(v_red, 1)
            vector.wait_ge(pe_sem, 1)
            vector.tensor_copy(s16[:], psum_s[:]).then_inc(v_sem, 1)
            vector.wait_ge(pe_sem, 2)
            vector.sem_clear(pe_sem)
            vector.tensor_copy(out_sb[:], psum_o[:]).then_inc(v_out, 1)

        @block.tensor
        def _(tensor):
            # s[d, b] += sum_p red_c[p, d] * mask[p, b], accumulated over
            # chunks; PE order interleaves gpsimd chunks at their expected
            # completion times
            dve_done = 0
            gp_done = 0
            order = DVE_CHUNKS[:5] + [GP_CHUNKS[0]] + [DVE_CHUNKS[5]] + \
                [GP_CHUNKS[1]] + DVE_CHUNKS[6:]
            assert sorted(order) == list(range(NCHUNK))
            for i, c in enumerate(order):
                if c in GP_CHUNKS:
                    gp_done += 1
                    tensor.wait_ge(g_red, gp_done)
                else:
                    dve_done += 1
                    tensor.wait_ge(v_red, dve_done)
                mm = tensor.matmul(
                    psum_s[:],
                    xh[c][:, :DIN],
                    mask_sb[:],
                    start=(i == 0),
                    stop=(i == NCHUNK - 1),
                )
            tensor.sem_clear(v_red)
            tensor.sem_clear(g_red)
            mm.then_inc(pe_sem, 1)
            tensor.wait_ge(v_w16, 1)
            tensor.sem_clear(v_w16)
            tensor.wait_ge(v_sem, 1)
            tensor.sem_clear(v_sem)
            # out[b, jd] = sum_d s[d, b] * W[d, jd]
            tensor.matmul(
                psum_o[:], s16[:], w16[:], start=True, stop=True
            ).then_inc(pe_sem, 1)

    return nc


def _get_nc():
    if "nc" not in _cache:
        _cache["nc"] = _build_nc()
    return _cache["nc"]


def _in_maps(x, W):
    x = np.ascontiguousarray(x, dtype=np.float32)
    W = np.ascontiguousarray(W, dtype=np.float32)
    return [{"x": x[i * BSH : (i + 1) * BSH], "W": W} for i in range(N_CORES)]


def kernel(x, W, **profile_kwargs):
    nc = _get_nc()
    res = run_bass_kernel_spmd(nc, _in_maps(x, W), list(range(N_CORES)), **profile_kwargs)
    out = np.concatenate([r["out"] for r in res.results], axis=0)
    ret = out.reshape(B, 10, 16).astype(np.float32)
    if profile_kwargs:
        ret = (ret, res)
    return ret


# revision 4
# speedup vs baseline: 1.1788x; 1.1788x over previous
"""Trainium2 Bass kernel for nn_Capsule_16484084482446.

Reference math collapses: with cw = softmax(rw, axis=1),
  outputs[b,j,d] = sum_i sum_n cw[b,i,n] * u[b,j,n,d]
                 = sum_n u[b,j,n,d]           (since sum_i cw[b,i,n] == 1)
so the routing loop is a no-op and the final result is
  out = (sum_n x[b,n,:]) @ W   reshaped to (B, 10, 16).

Kernel strategy (data-parallel over batch, 4 batches per core x 8 cores):
  per core: x_shard (4, 4096, 128) viewed as 128 partitions x (128 rows x 128 d);
  partition p holds rows [128p, 128p+128), so batch b owns partitions [32b, 32b+32).

v2c (baseline fp32 at 39.9us; measured-trace findings baked in):
  1. x chunks stream on the single sync HWDGE queue IN ORDER: the 16 DMA
     engines saturate (~375 GB/s) either way, and a single queue makes
     chunk c complete at its stream-position time - splitting chunks
     across both HWDGE queues halved each queue's rate and DOUBLED every
     chunk's completion latency (v2b regression, +2.7us).
  2. W loads via the scalar HWDGE queue at t=0: its 80KB ride the engine
     ramp-up while the sync queue's first descriptors generate, instead
     of stealing x bandwidth at the end of the stream (baseline).
  3. DVE fold level 1 writes fp16 (fp32+fp32->fp16 add, measured 690ns
     per 512-elem level); levels 2+ run at the 16-bit 2-elem/cycle rate
     (415/283/215ns vs fp32 684/418/284). Fold work ~21us -> ~15.8us,
     under the 21.4us stream window, so folds track the stream with no
     end-of-stream backlog (baseline lost ~3us here).
     gpsimd folding does NOT help: measured 2.5x slower than DVE and its
     SBUF port pressure slowed concurrent DVE ops up to 4x (v2b).
  4. All matmuls fp16 single-pass: mask-matmul LDWEIGHTS+MATMUL measured
     105+164ns vs fp32 dual-pass 625+700ns; final W-matmul 80+292ns.
     PSUM accumulates fp32.
  5. Chunk sizes taper 8->4->2->2 at the end so the final fold is 276ns.
  6. No wait on the out-DMA completion semaphore: the block-exit DRAIN
     (measured 160ns) flushes the queue, overlapping the exit barrier
     with the out-DMA flight (saves the ~1.2us land+sem wait).
  Accuracy: fp16 folds of ~N(0,1) data with fp32 PSUM accumulation ->
  rel err ~6e-4 measured (gate 2e-2; inputs are seeded the same locally
  and in the harness, so a local pass is deterministic).

Raw Bass (no TileContext): Tile's tail drain needs more sync-wait slots than the
TRN2 CTRL encoding allows for this DMA-lane mix, and its end-of-kernel barriers
would dominate a ~36 us kernel. Every semaphore is cleared by its final consumer
right after its last wait, so the NEFF re-executes cleanly (profilers loop it).
"""

from contextlib import ExitStack

import numpy as np

import concourse.bass as bass
from concourse import mybir
from concourse.bass_utils import run_bass_kernel_spmd

N_CORES = 8
B, N, DIN = 32, 4096, 128
BSH = B // N_CORES          # 4 batches per core
DOUT = 160                  # 10 capsules * 16 dims
# rows-per-partition split: 8-row ramp (early DVE start), 16-row middle,
# tapered tail so the last folds are tiny
CHUNKS = [8, 8, 16, 16, 16, 16, 16, 8, 8, 8, 4, 2, 2]
assert sum(CHUNKS) == BSH * N // 128
NCHUNK = len(CHUNKS)

F32 = mybir.dt.float32
F16 = mybir.dt.float16

_cache = {}


def _fold(eng, xc_c, xh_c, rows):
    """Halving fold of xc_c (fp32, rows*DIN) into xh_c[:, :DIN] (fp16).
    Level 1 casts fp32->fp16; later levels run at the 2x 16-bit rate."""
    s = rows // 2
    op = eng.tensor_add(
        xh_c[:, : s * DIN], xc_c[:, : s * DIN], xc_c[:, s * DIN :]
    )
    while s > 1:
        s //= 2
        op = eng.tensor_add(
            xh_c[:, : s * DIN],
            xh_c[:, : s * DIN],
            xh_c[:, s * DIN : 2 * s * DIN],
        )
    return op


def _build_nc(chunks=None, out_wait=False):
    global CHUNKS, NCHUNK
    if chunks is not None:
        CHUNKS = chunks
        NCHUNK = len(CHUNKS)
    assert sum(CHUNKS) == BSH * N // 128
    nc = bass.Bass()
    x = nc.dram_tensor("x", [BSH, N, DIN], F32, kind="ExternalInput")
    w = nc.dram_tensor("W", [DIN, DOUT], F32, kind="ExternalInput")
    out = nc.dram_tensor("out", [BSH, DOUT], F32, kind="ExternalOutput")

    # (128, 128, 128): partition p, row-in-partition n, feature d
    x3 = x[:].flatten_outer_dims().rearrange("(p n) d -> p n d", p=128)
    starts = np.cumsum([0] + CHUNKS).tolist()

    with ExitStack() as ctx:
        ec = ctx.enter_context
        xc = [ec(nc.sbuf_tensor(f"xc{c}", [128, CHUNKS[c] * DIN], F32))
              for c in range(NCHUNK)]
        xh = [ec(nc.sbuf_tensor(f"xh{c}", [128, (CHUNKS[c] // 2) * DIN], F16))
              for c in range(NCHUNK)]
        w_sb = ec(nc.sbuf_tensor("w_sb", [DIN, DOUT], F32))
        w16 = ec(nc.sbuf_tensor("w16", [DIN, DOUT], F16))
        mask_sb = ec(nc.sbuf_tensor("mask_sb", [128, BSH], F16))
        s16 = ec(nc.sbuf_tensor("s16", [DIN, BSH], F16))
        out_sb = ec(nc.sbuf_tensor("out_sb", [BSH, DOUT], F32))
        psum_s = ec(nc.psum_tensor("psum_s", [DIN, BSH], F32))
        psum_o = ec(nc.psum_tensor("psum_o", [BSH, DOUT], F32))

        dma_w = ec(nc.semaphore("dma_w"))
        dma_c = [ec(nc.semaphore(f"dma_c{c}")) for c in range(NCHUNK)]
        v_red = ec(nc.semaphore("v_red"))    # +1 per finished DVE fold
        v_w16 = ec(nc.semaphore("v_w16"))    # w16 ready
        pe_sem = ec(nc.semaphore("pe_sem"))
        v_sem = ec(nc.semaphore("v_sem"))    # s16 ready
        v_out = ec(nc.semaphore("v_out"))
        dma_out = ec(nc.semaphore("dma_out"))  # never waited (drain flushes)
        # Sem hygiene without an entry barrier: every semaphore is cleared by
        # its final consumer right after the consumer's last wait on it, so
        # every run (the profiler re-executes the NEFF) starts from zeros.
        # dma_out only ever grows; nothing waits on an absolute value.
        block = ec(nc.Block())

        @block.sync
        def _(sync):
            for c in range(NCHUNK):
                sync.dma_start(
                    xc[c][:], x3[:, starts[c] : starts[c + 1], :]
                ).then_inc(dma_c[c], 16)
            sync.wait_ge(v_out, 1)
            sync.sem_clear(v_out)
            sync.dma_start(out[:], out_sb[:]).then_inc(dma_out, 16)
            if out_wait:
                sync.wait_ge(dma_out, 16)
                sync.sem_clear(dma_out)

        @block.scalar
        def _(scalar):
            # W rides the engine ramp-up on the second HWDGE queue while the
            # sync queue's first x descriptors are still generating
            scalar.dma_start(w_sb[:], w[:]).then_inc(dma_w, 16)

        @block.vector
        def _(vector):
            # 0/1 batch mask, one 32-partition quadrant at a time (nonzero
            # partition bases only allow 32-partition windows)
            for q in range(4):
                for b in range(BSH):
                    vector.memset(
                        mask_sb[32 * q : 32 * (q + 1), b : b + 1],
                        1.0 if q == b else 0.0,
                    )
            vector.wait_ge(dma_w, 16)
            vector.sem_clear(dma_w)
            vector.tensor_copy(w16[:], w_sb[:]).then_inc(v_w16, 1)
            for c in range(NCHUNK):
                vector.wait_ge(dma_c[c], 16)
                vector.sem_clear(dma_c[c])
                _fold(vector, xc[c], xh[c], CHUNKS[c]).then_inc(v_red, 1)
            vector.wait_ge(pe_sem, 1)
            vector.tensor_copy(s16[:], psum_s[:]).then_inc(v_sem, 1)
            vector.wait_ge(pe_sem, 2)
            vector.sem_clear(pe_sem)
            vector.tensor_copy(out_sb[:], psum_o[:]).then_inc(v_out, 1)

        @block.tensor
        def _(tensor):
            # s[d, b] += sum_p red_c[p, d] * mask[p, b], accumulated over chunks
            for c in range(NCHUNK):
                tensor.wait_ge(v_red, c + 1)
                mm = tensor.matmul(
                    psum_s[:],
                    xh[c][:, :DIN],
                    mask_sb[:],
                    start=(c == 0),
                    stop=(c == NCHUNK - 1),
                )
            tensor.sem_clear(v_red)
            mm.then_inc(pe_sem, 1)
            tensor.wait_ge(v_w16, 1)
            tensor.sem_clear(v_w16)
            tensor.wait_ge(v_sem, 1)
            tensor.sem_clear(v_sem)
            # out[b, jd] = sum_d s[d, b] * W[d, jd]
            tensor.matmul(
                psum_o[:], s16[:], w16[:], start=True, stop=True
            ).then_inc(pe_sem, 1)

    return nc


def _get_nc():
    if "nc" not in _cache:
        _cache["nc"] = _build_nc()
    return _cache["nc"]


def _in_maps(x, W):
    x = np.ascontiguousarray(x, dtype=np.float32)
    W = np.ascontiguousarray(W, dtype=np.float32)
    return [{"x": x[i * BSH : (i + 1) * BSH], "W": W} for i in range(N_CORES)]


def kernel(x, W, **profile_kwargs):
    nc = _get_nc()
    res = run_bass_kernel_spmd(nc, _in_maps(x, W), list(range(N_CORES)), **profile_kwargs)
    out = np.concatenate([r["out"] for r in res.results], axis=0)
    ret = out.reshape(B, 10, 16).astype(np.float32)
    if profile_kwargs:
        ret = (ret, res)
    return ret
